# revision 13
# baseline (speedup 1.0000x reference)
"""Trainium2 Bass kernel for the PrimedGKA layer (gated linear attention with
Chebyshev query refinement), tensor-parallel over the 16 query heads across
8 NeuronCores (2 q-heads + their shared kv-head per core), out-projection
computed per-core against the core's Wo row-block; partial outputs summed on
the host (unshard of the sum-sharded output).

Self-contained: hardcodes all shapes from the problem spec.
"""
import numpy as np

B, T, D = 1, 1024, 1024
HQ, HKV, HK, HV = 16, 4, 64, 64
KW = 4
NCORES = 8
L = 128                 # chunk length
NCH = T // L            # 8 chunks
CHEB_DAMP = 0.25
EPS = 1e-6
QSCALE = HK ** -0.5

_PROG_CACHE = {}


def _build_program(dbg=False, reps=1):
    import concourse.bacc as bacc
    import concourse.mybir as mybir
    from concourse.tile import TileContext

    dt = mybir.dt
    f32 = dt.float32
    f32r = dt.float32r
    AF = mybir.ActivationFunctionType
    ALU = mybir.AluOpType
    X = mybir.AxisListType.X

    nc = bacc.Bacc("TRN2", target_bir_lowering=False, debug=False,
                   num_devices=NCORES)

    xT = nc.dram_tensor("xT", [D, T], f32, kind="ExternalInput")
    wcat = nc.dram_tensor("wcat", [D, 261], f32, kind="ExternalInput")
    wo = nc.dram_tensor("wo", [128, D], f32, kind="ExternalInput")
    wcv = nc.dram_tensor("wcv", [256, KW], f32, kind="ExternalInput")
    alog = nc.dram_tensor("alog", [1, 2], f32, kind="ExternalInput")
    dtb = nc.dram_tensor("dtb", [1, 2], f32, kind="ExternalInput")
    iden = nc.dram_tensor("iden", [128, 128], f32, kind="ExternalInput")
    umask = nc.dram_tensor("umask", [128, 128], f32, kind="ExternalInput")
    outp = nc.dram_tensor("outp", [T, D], f32, kind="ExternalOutput")
    if dbg:
        dqkv = nc.dram_tensor("dqkv", [2, 128, T], f32, kind="ExternalOutput")
        dgate = nc.dram_tensor("dgate", [5, T], f32, kind="ExternalOutput")
        dkvtm = nc.dram_tensor("dkvtm", [NCH, 128, 128], f32, kind="ExternalOutput")
        dgt = nc.dram_tensor("dgt", [NCH, 128, 256], f32, kind="ExternalOutput")
        dhm = nc.dram_tensor("dhm", [NCH, 2, 64, 128], f32, kind="ExternalOutput")
        don = nc.dram_tensor("don", [NCH, 128, 128], f32, kind="ExternalOutput")

    def r32(ap):
        return ap  # fp32 for now; fast-dtype pass comes later

    with TileContext(nc) as tc:
      import contextlib
      for _rep in range(reps):
        ctx = contextlib.ExitStack()
        with ctx:
            pers = ctx.enter_context(tc.tile_pool(name="pers", bufs=1))
            p_gl = ctx.enter_context(tc.tile_pool(name="p_gl", bufs=4))
            p_st = ctx.enter_context(tc.tile_pool(name="p_st", bufs=8))
            p_big = ctx.enter_context(tc.tile_pool(name="p_big", bufs=4))
            p_gt = ctx.enter_context(tc.tile_pool(name="p_gt", bufs=4))
            p_gb = ctx.enter_context(tc.tile_pool(name="p_gb", bufs=4))
            p_sm = ctx.enter_context(tc.tile_pool(name="p_sm", bufs=4))
            p_kv = ctx.enter_context(tc.tile_pool(name="p_kv", bufs=4))
            p_hm = ctx.enter_context(tc.tile_pool(name="p_hm", bufs=6))
            p_xq = ctx.enter_context(tc.tile_pool(name="p_xq", bufs=4))
            p_out = ctx.enter_context(tc.tile_pool(name="p_out", bufs=2))
            ps_pj = ctx.enter_context(tc.tile_pool(name="ps_pj", bufs=2, space="PSUM"))
            ps_big = ctx.enter_context(tc.tile_pool(name="ps_big", bufs=2, space="PSUM"))
            ps_med = ctx.enter_context(tc.tile_pool(name="ps_med", bufs=2, space="PSUM"))
            ps_sm = ctx.enter_context(tc.tile_pool(name="ps_sm", bufs=2, space="PSUM"))

            # ---- persistent loads ----
            xt_sb = pers.tile([128, 8, T], f32)
            nc.sync.dma_start(out=xt_sb[:], in_=xT[:].rearrange("(a p) t -> p a t", p=128))
            wcat_sb = pers.tile([128, 8, 261], f32)
            nc.sync.dma_start(out=wcat_sb[:], in_=wcat[:].rearrange("(a p) c -> p a c", p=128))
            wo_sb = pers.tile([128, D], f32)
            nc.sync.dma_start(out=wo_sb[:], in_=wo[:])
            wcv_sb = pers.tile([128, 2, KW], f32)
            nc.sync.dma_start(out=wcv_sb[:], in_=wcv[:].rearrange("(a p) k -> p a k", p=128))
            alog_sb = pers.tile([1, 2], f32)
            nc.sync.dma_start(out=alog_sb[:], in_=alog[:])
            dtb_sb = pers.tile([1, 2], f32)
            nc.sync.dma_start(out=dtb_sb[:], in_=dtb[:])
            iden_sb = pers.tile([128, 128], f32)
            nc.sync.dma_start(out=iden_sb[:], in_=iden[:])
            um_sb = pers.tile([128, 128], f32)
            nc.sync.dma_start(out=um_sb[:], in_=umask[:])

            ones128 = pers.tile([1, 128], f32)
            nc.vector.memset(ones128[:], 1.0)
            zeros_hm = pers.tile([64, 128], f32)
            nc.vector.memset(zeros_hm[:], 0.0)
            epsb = pers.tile([128, 1], f32)
            nc.vector.memset(epsb[:], EPS)

            Rq = pers.tile([128, T], f32)     # raw q projection (pre-conv), fm
            Rkv = pers.tile([128, T], f32)    # raw k|v projection, fm
            gates_sb = pers.tile([5, T], f32)  # raw gate projections, fm
            Cq = pers.tile([128, T], f32)
            Ckv = pers.tile([128, T], f32)
            Sq = pers.tile([128, T], f32)     # silu(conv(q)), fm
            Skv = pers.tile([128, T], f32)    # silu(conv(k|v)), fm
            qst = pers.tile([64, 2 * T], f32)  # q heads stacked per chunk, scaled

            # ---- projections: qkvT[c, t] = sum_d W[d, c] xT[d, t] ----
            for ct, (m, dst) in enumerate(((128, Rq), (128, Rkv), (5, gates_sb))):
                c0 = ct * 128
                for th in range(2):
                    ps = ps_pj.tile([128, 512], f32, tag="pj")
                    for d in range(8):
                        nc.tensor.matmul(
                            ps[0:m, :],
                            r32(wcat_sb[:, d, c0:c0 + m]),
                            r32(xt_sb[:, d, th * 512:(th + 1) * 512]),
                            start=(d == 0), stop=(d == 7),
                        )
                    nc.vector.tensor_copy(dst[0:m, th * 512:(th + 1) * 512], ps[0:m, :])

            # ---- causal depthwise conv (shifts along free/time dim) + silu ----
            for ct, (R, C, S) in enumerate(((Rq, Cq, Sq), (Rkv, Ckv, Skv))):
                w = lambda k: wcv_sb[:, ct, k:k + 1]
                nc.vector.tensor_scalar(C[:, 0:T], R[:, 0:T], w(3), None, ALU.mult)
                for tap, sh in ((2, 1), (1, 2), (0, 3)):
                    nc.vector.scalar_tensor_tensor(
                        C[:, sh:T], R[:, 0:T - sh], w(tap), C[:, sh:T],
                        op0=ALU.mult, op1=ALU.add)
                nc.scalar.activation(S[:], C[:], AF.Silu)

            # ---- per-head gate constants broadcast over 128 partitions ----
            # gate math runs time-major per chunk (head on the free axis, since
            # engine operands must start at partition 0/32/64/96)
            era = pers.tile([1, 2], f32)
            nc.scalar.activation(era[:], alog_sb[:], AF.Exp)
            erd = pers.tile([1, 2], f32)
            nc.scalar.activation(erd[:], dtb_sb[:], AF.Exp)
            ps_bc = ps_sm.tile([128, 128], f32, tag="psm")
            nc.tensor.matmul(ps_bc[:, 0:2], ones128[:], era[:], start=True, stop=True)
            negea_bc = pers.tile([128, 2], f32)
            nc.vector.tensor_scalar(negea_bc[:], ps_bc[:, 0:2], -1.0, None, ALU.mult)
            ps_bc2 = ps_sm.tile([128, 128], f32, tag="psm")
            nc.tensor.matmul(ps_bc2[:, 0:2], ones128[:], erd[:], start=True, stop=True)
            edtb_bc = pers.tile([128, 2], f32)
            nc.vector.tensor_copy(edtb_bc[:], ps_bc2[:, 0:2])

            # ---- stacked, scaled q:  qst[f, ci*256 + h*128 + t] ----
            qv = qst[:].rearrange("p (c h t) -> p c h t", c=NCH, h=2)
            for h in range(2):
                nc.vector.tensor_scalar(
                    qv[:, :, h, :],
                    Sq[h * 64:(h + 1) * 64, :].rearrange("p (c t) -> p c t", c=NCH),
                    QSCALE, None, ALU.mult)

            if dbg and _rep == 0:
                nc.sync.dma_start(out=dqkv[0], in_=Sq[:])
                nc.sync.dma_start(out=dqkv[1], in_=Skv[:])
                nc.sync.dma_start(out=dgate[:], in_=gates_sb[:])

            # ---- chunked recurrence ----
            hm_prev = [zeros_hm, zeros_hm]
            for ci in range(NCH):
                s = slice(ci * L, (ci + 1) * L)

                # time-major gate math: transpose the 5 raw gate rows, then
                # softplus/sigmoid synthesized from Exp/Ln, cumsum via U-mask matmul
                ps_gtm = ps_sm.tile([128, 128], f32, tag="psm")
                nc.tensor.transpose(ps_gtm[:, 0:5], gates_sb[:, s], iden_sb[0:5, 0:5])
                gtm = p_sm.tile([128, 5], f32, tag="gtm")
                nc.vector.tensor_copy(gtm[:], ps_gtm[:, 0:5])
                e_a = p_sm.tile([128, 2], f32, tag="e_a")
                nc.scalar.activation(e_a[:], gtm[:, 0:2], AF.Exp)
                e_a2 = p_sm.tile([128, 2], f32, tag="e_a2")
                nc.vector.tensor_tensor(e_a2[:], e_a[:], edtb_bc[:], ALU.mult)
                sp_tm = p_sm.tile([128, 2], f32, tag="sp_tm")
                nc.scalar.activation(sp_tm[:], e_a2[:], AF.Ln, bias=1.0)
                g_tm = p_sm.tile([128, 2], f32, tag="g_tm")
                nc.vector.tensor_tensor(g_tm[:], sp_tm[:], negea_bc[:], ALU.mult)
                ps_G = ps_sm.tile([128, 128], f32, tag="psm")
                nc.tensor.matmul(ps_G[:, 0:2], um_sb[:], g_tm[:], start=True, stop=True)
                G_sb = p_sm.tile([128, 2], f32, tag="G_sb")
                nc.vector.tensor_copy(G_sb[:], ps_G[:, 0:2])
                e_g = p_sm.tile([128, 3], f32, tag="e_g")
                nc.scalar.activation(e_g[:], gtm[:, 2:5], AF.Exp, scale=-1.0)
                d_g = p_sm.tile([128, 3], f32, tag="d_g")
                nc.vector.tensor_scalar(d_g[:], e_g[:], 1.0, None, ALU.add)
                ab_tm = p_sm.tile([128, 3], f32, tag="ab_tm")
                nc.vector.reciprocal(ab_tm[:], d_g[:])

                # G rows (via column transposes) -> gamma row, Gamma^T mask
                growcat = p_gl.tile([1, 256], f32, tag="growcat")
                ps_r = ps_sm.tile([128, 128], f32, tag="psm")
                nc.tensor.transpose(ps_r[0:1, :], G_sb[:, 0:1], iden_sb[:])
                nc.vector.tensor_copy(growcat[0:1, 0:L], ps_r[0:1, 0:L])
                ps_r2 = ps_sm.tile([128, 128], f32, tag="psm")
                nc.tensor.transpose(ps_r2[0:1, :], G_sb[:, 1:2], iden_sb[:])
                nc.vector.tensor_copy(growcat[0:1, L:2 * L], ps_r2[0:1, 0:L])
                gamrow = p_gl.tile([1, 256], f32, tag="gamrow")
                nc.scalar.activation(gamrow[:], growcat[:], AF.Exp)

                # GammaT[s, h*L + t] = exp(G_t - G_s) for s<=t else 0
                ps_GB = ps_big.tile([128, 256], f32, tag="pbig")
                nc.tensor.matmul(ps_GB[:], ones128[:], growcat[:], start=True, stop=True)
                dm = p_big.tile([128, 256], f32, tag="dm")
                for h in range(2):
                    nc.vector.scalar_tensor_tensor(
                        dm[:, h * L:(h + 1) * L], ps_GB[:, h * L:(h + 1) * L], 1.0,
                        G_sb[:, h:h + 1].broadcast_to([128, L]),
                        op0=ALU.mult, op1=ALU.subtract)
                um2 = um_sb[:].unsqueeze(1).broadcast_to([128, 2, L])
                dm2 = p_big.tile([128, 256], f32, tag="dm2")
                nc.vector.tensor_tensor(dm2[:].rearrange("p (h t) -> p h t", h=2),
                                        dm[:].rearrange("p (h t) -> p h t", h=2),
                                        um2, ALU.mult)
                em = p_big.tile([128, 256], f32, tag="em")
                nc.scalar.activation(em[:], dm2[:], AF.Exp)
                gt_sb = p_gt.tile([128, 256], f32, tag="gt")
                nc.vector.tensor_tensor(gt_sb[:].rearrange("p (h t) -> p h t", h=2),
                                        em[:].rearrange("p (h t) -> p h t", h=2),
                                        um2, ALU.mult)

                # gammaB[f, h*L + t] = gamma_t(h)  (broadcast over 64 partitions)
                ps_gb = ps_med.tile([64, 256], f32, tag="pmed")
                nc.tensor.matmul(ps_gb[:], ones128[0:1, 0:64], gamrow[:], start=True, stop=True)
                gb_sb = p_gb.tile([64, 256], f32, tag="gb")
                nc.vector.tensor_copy(gb_sb[:], ps_gb[:])

                # k/v transposes to time-major; k l2-normalized, v beta-scaled
                ps_kt = ps_sm.tile([128, 128], f32, tag="psm")
                nc.tensor.transpose(ps_kt[:, 0:64], Skv[0:64, s], iden_sb[0:64, 0:64])
                sqk = p_sm.tile([128, 64], f32, tag="sqk")
                nc.scalar.activation(sqk[:], ps_kt[:, 0:64], AF.Square)
                ssk = p_sm.tile([128, 1], f32, tag="ssk")
                nc.vector.tensor_reduce(ssk[:], sqk[:], X, ALU.add)
                lnk = p_sm.tile([128, 1], f32, tag="lnk")
                nc.scalar.activation(lnk[:], ssk[:], AF.Ln)
                nrk = p_sm.tile([128, 1], f32, tag="nrk")
                nc.scalar.activation(nrk[:], lnk[:], AF.Exp, scale=0.5)
                nre = p_sm.tile([128, 1], f32, tag="nre")
                nc.vector.tensor_scalar(nre[:], nrk[:], EPS, None, ALU.add)
                invk = p_sm.tile([128, 1], f32, tag="invk")
                nc.vector.reciprocal(invk[:], nre[:])
                kv_tm = p_kv.tile([128, 128], f32, tag="kvtm")
                nc.vector.tensor_scalar(kv_tm[:, 0:64], ps_kt[:, 0:64], invk[:], None, ALU.mult)
                ps_vt = ps_sm.tile([128, 128], f32, tag="psm")
                nc.tensor.transpose(ps_vt[:, 0:64], Skv[64:128, s], iden_sb[64:128, 64:128])
                nc.vector.tensor_scalar(kv_tm[:, 64:128], ps_vt[:, 0:64],
                                        ab_tm[:, 2:3], None, ALU.mult)

                # normalized k back to feature-major
                ps_kf = ps_sm.tile([128, 128], f32, tag="psm")
                nc.tensor.transpose(ps_kf[0:64, :], kv_tm[:, 0:64], iden_sb[:])
                kfn = p_kv.tile([64, 128], f32, tag="kfn")
                nc.vector.tensor_copy(kfn[:], ps_kf[0:64, :])

                # decay-weighted keys for the state update
                kw_sb = p_kv.tile([128, 128], f32, tag="kw")
                for h in range(2):
                    nc.vector.tensor_scalar(kw_sb[:, h * 64:(h + 1) * 64], kv_tm[:, 0:64],
                                            gt_sb[:, h * L + L - 1:h * L + L], None, ALU.mult)

                # state update: HM_h <- gammaL * HM_h + Kw_h^T [K | V]
                hm_new = []
                for h in range(2):
                    ps_hm = ps_sm.tile([128, 128], f32, tag="psm")
                    nc.tensor.matmul(ps_hm[0:64, :], kw_sb[:, h * 64:(h + 1) * 64],
                                     kv_tm[:], start=True, stop=True)
                    hmsc = p_hm.tile([64, 128], f32, tag="hmsc")
                    nc.scalar.activation(hmsc[:], hm_prev[h][:], AF.Copy,
                                         scale=gb_sb[:, h * L + L - 1:h * L + L])
                    hm = p_hm.tile([64, 128], f32, tag="hm")
                    nc.vector.tensor_tensor(hm[:], ps_hm[0:64, :], hmsc[:], ALU.add)
                    hm_new.append(hm)
                    if dbg:
                        nc.sync.dma_start(out=dhm[ci, h], in_=hm[:])

                if dbg and _rep == 0:
                    nc.sync.dma_start(out=dkvtm[ci], in_=kv_tm[:])
                    nc.sync.dma_start(out=dgt[ci], in_=gt_sb[:])

                # ---- three operator applications (2x H-refine, 1x M-output) ----
                qchunk = qst[:, ci * 256:(ci + 1) * 256]
                xcur = qchunk
                for it in range(2):
                    xg = p_xq.tile([64, 256], f32, tag="xg")
                    nc.vector.tensor_tensor(xg[:], xcur, gb_sb[:], ALU.mult)
                    ps_p = ps_big.tile([128, 256], f32, tag="pbig")
                    nc.tensor.matmul(ps_p[:], r32(kfn[:]), r32(xcur), start=True, stop=True)
                    a_sb = p_big.tile([128, 256], f32, tag="a")
                    nc.vector.tensor_tensor(a_sb[:], ps_p[:], gt_sb[:], ALU.mult)
                    ps_y = ps_med.tile([64, 256], f32, tag="pmed")
                    nc.tensor.matmul(ps_y[:], r32(kv_tm[:, 0:64]), r32(a_sb[:]),
                                     start=True, stop=False)
                    for h in range(2):
                        nc.tensor.matmul(ps_y[:, h * L:(h + 1) * L],
                                         hm_prev[h][:, 0:64], xg[:, h * L:(h + 1) * L],
                                         start=False, stop=True)
                    xq = p_xq.tile([64, 256], f32, tag="xq")
                    nc.vector.scalar_tensor_tensor(xq[:], ps_y[:], -CHEB_DAMP, qchunk,
                                                   op0=ALU.mult, op1=ALU.add)
                    xcur = xq[:]

                xg2 = p_xq.tile([64, 256], f32, tag="xg")
                nc.vector.tensor_tensor(xg2[:], xcur, gb_sb[:], ALU.mult)
                ps_p = ps_big.tile([128, 256], f32, tag="pbig")
                nc.tensor.matmul(ps_p[:], r32(kfn[:]), r32(xcur), start=True, stop=True)
                a_sb = p_big.tile([128, 256], f32, tag="a")
                nc.vector.tensor_tensor(a_sb[:], ps_p[:], gt_sb[:], ALU.mult)
                ps_o = ps_sm.tile([128, 128], f32, tag="psm")
                for h in range(2):
                    nc.tensor.matmul(ps_o[:, h * 64:(h + 1) * 64],
                                     a_sb[:, h * L:(h + 1) * L], kv_tm[:, 64:128],
                                     start=True, stop=False)
                    nc.tensor.matmul(ps_o[:, h * 64:(h + 1) * 64],
                                     xg2[:, h * L:(h + 1) * L], hm_prev[h][:, 64:128],
                                     start=False, stop=True)

                hm_prev = hm_new

                # ---- alpha gate + per-head rmsnorm (time-major) ----
                al2 = ab_tm[:, 0:2].unsqueeze(2).broadcast_to([128, 2, 64])
                oa = p_out.tile([128, 128], f32, tag="oa")
                nc.vector.tensor_tensor(oa[:].rearrange("p (h v) -> p h v", h=2),
                                        ps_o[:].rearrange("p (h v) -> p h v", h=2),
                                        al2, ALU.mult)
                sqo = p_out.tile([128, 128], f32, tag="sqo")
                nc.scalar.activation(sqo[:], oa[:], AF.Square)
                sso = p_sm.tile([128, 2], f32, tag="sso")
                nc.vector.tensor_reduce(sso[:], sqo[:].rearrange("p (h v) -> p h v", h=2),
                                        X, ALU.add)
                lno = p_sm.tile([128, 2], f32, tag="lno")
                nc.scalar.activation(lno[:], sso[:], AF.Ln, bias=epsb[:], scale=1.0 / 64.0)
                rmso = p_sm.tile([128, 2], f32, tag="rmso")
                nc.scalar.activation(rmso[:], lno[:], AF.Exp, scale=0.5)
                invo = p_sm.tile([128, 2], f32, tag="invo")
                nc.vector.reciprocal(invo[:], rmso[:])
                on = p_out.tile([128, 128], f32, tag="on")
                nc.vector.tensor_tensor(on[:].rearrange("p (h v) -> p h v", h=2),
                                        oa[:].rearrange("p (h v) -> p h v", h=2),
                                        invo[:].unsqueeze(2).broadcast_to([128, 2, 64]),
                                        ALU.mult)
                if dbg and _rep == 0:
                    nc.sync.dma_start(out=don[ci], in_=on[:])

                # ---- transpose to feature-major + out-projection row block ----
                ps_of = ps_sm.tile([128, 128], f32, tag="psm")
                nc.tensor.transpose(ps_of[:], on[:], iden_sb[:])
                ofm = p_out.tile([128, 128], f32, tag="ofm")
                nc.vector.tensor_copy(ofm[:], ps_of[:])
                out_sb = p_out.tile([128, D], f32, tag="outsb")
                for nh in range(2):
                    ps_out = ps_pj.tile([128, 512], f32, tag="pj")
                    nc.tensor.matmul(ps_out[:], r32(ofm[:]),
                                     r32(wo_sb[:, nh * 512:(nh + 1) * 512]),
                                     start=True, stop=True)
                    nc.vector.tensor_copy(out_sb[:, nh * 512:(nh + 1) * 512], ps_out[:])
                nc.sync.dma_start(out=outp[s, :], in_=out_sb[:])

    # The act-table placement pass maps each activation func to the FIRST
    # table containing it; Exp->exp_and_others and Ln->natural_log would then
    # thrash with a table reload on every Exp<->Ln alternation. Compile with
    # natural_log_exp_and_others (has both) hoisted to the front, then remap
    # the emitted set ids back to the real act_info.json indices.
    import concourse.bacc as bacc_mod
    from concourse.hw_specs import get_activation_tables as _gat
    orig_tables = _gat(nc.m.arch)
    orig_names = list(orig_tables.keys())
    pref = "natural_log_exp_and_others"
    reordered = {pref: orig_tables[pref],
                 **{k: v for k, v in orig_tables.items() if k != pref}}
    pnames = list(reordered.keys())
    bacc_mod.get_activation_tables = lambda arch: reordered
    try:
        nc.compile()
    finally:
        bacc_mod.get_activation_tables = _gat
    for b in nc.main_func.blocks:
        for i in b.instructions:
            if isinstance(i, mybir.InstLoadActFuncSet):
                i.act_func_set_id = orig_names.index(pnames[i.act_func_set_id])
    return nc


def _prep_core_inputs(c, x, Wq, Wk, Wv, Wconv, Wa, Walpha, Wb, A_log, dt_bias,
                      norm_w, Wo, xT, iden, um):
    f32 = np.float32
    h0, h1, hk = 2 * c, 2 * c + 1, c // 2
    wcat = np.hstack([
        Wq[:, h0 * HK:(h0 + 1) * HK], Wq[:, h1 * HK:(h1 + 1) * HK],
        Wk[:, hk * HK:(hk + 1) * HK], Wv[:, hk * HV:(hk + 1) * HV],
        Wa[:, h0:h0 + 1], Wa[:, h1:h1 + 1],
        Walpha[:, h0:h0 + 1], Walpha[:, h1:h1 + 1],
        Wb[:, hk:hk + 1],
    ]).astype(f32)
    qoff, koff, voff = 0, HQ * HK, HQ * HK + HKV * HK
    wcv = np.vstack([
        Wconv[qoff + h0 * HK: qoff + (h0 + 1) * HK],
        Wconv[qoff + h1 * HK: qoff + (h1 + 1) * HK],
        Wconv[koff + hk * HK: koff + (hk + 1) * HK],
        Wconv[voff + hk * HV: voff + (hk + 1) * HV],
    ]).astype(f32)
    wo_scale = np.tile(np.asarray(norm_w, f32), HQ)
    Wo_s = np.asarray(Wo, f32) * wo_scale[:, None]
    wo = np.ascontiguousarray(
        np.vstack([Wo_s[h0 * HV:(h0 + 1) * HV], Wo_s[h1 * HV:(h1 + 1) * HV]])).astype(f32)
    alog = np.asarray(A_log, f32)[[h0, h1]].reshape(2, 1).copy()
    dtb = np.asarray(dt_bias, f32)[[h0, h1]].reshape(2, 1).copy()
    return dict(xT=xT, wcat=np.ascontiguousarray(wcat), wo=wo,
                wcv=np.ascontiguousarray(wcv), alog=alog, dtb=dtb,
                iden=iden, umask=um)


def make_in_maps(x, Wq, Wk, Wv, Wconv, Wa, Walpha, Wb, A_log, dt_bias, norm_w, Wo):
    f32 = np.float32
    x2 = np.asarray(x, f32).reshape(T, D)
    xT = np.ascontiguousarray(x2.T)
    iden = np.eye(128, dtype=f32)
    um = np.ascontiguousarray(np.triu(np.ones((128, 128), f32)))
    args = (x, np.asarray(Wq, f32), np.asarray(Wk, f32), np.asarray(Wv, f32),
            np.asarray(Wconv, f32), np.asarray(Wa, f32), np.asarray(Walpha, f32),
            np.asarray(Wb, f32), A_log, dt_bias, norm_w, Wo)
    return [_prep_core_inputs(c, *args, xT=xT, iden=iden, um=um)
            for c in range(NCORES)]


def get_program(dbg=False, reps=1):
    key = (dbg, reps)
    if key not in _PROG_CACHE:
        _PROG_CACHE[key] = _build_program(dbg, reps)
    return _PROG_CACHE[key]


def kernel(**inputs) -> np.ndarray:
    from concourse.bass_utils import run_bass_kernel_spmd
    nc = get_program(dbg=False)
    in_maps = make_in_maps(**inputs)
    res = run_bass_kernel_spmd(nc, in_maps, list(range(NCORES)))
    out = np.zeros((T, D), np.float32)
    for c in range(NCORES):
        out += res.results[c]["outp"]
    return out.reshape(B, T, D)


# revision 30
# speedup vs baseline: 2.7786x; 2.7786x over previous
"""Trainium2 Bass kernel for the PrimedGKA layer (gated linear attention with
Chebyshev query refinement), tensor-parallel over the 16 query heads across
8 NeuronCores (2 q-heads + their shared kv-head per core), out-projection
computed per-core against the core's Wo row-block; partial outputs summed on
the host (unshard of the sum-sharded output).

Precision plan: q/k/v datapath and all large matmuls in fp16 (PE runs 2-byte
matmuls at 1 cyc/row vs 4 for fp32) with fp32 PSUM accumulation; the decay
path (gate logits, cumulative log-decay G, the exp(G_t - G_s) mask build) and
the recurrent states stay fp32.

Self-contained: hardcodes all shapes from the problem spec.
"""
import numpy as np

B, T, D = 1, 1024, 1024
HQ, HKV, HK, HV = 16, 4, 64, 64
KW = 4
NCORES = 8
L = 128                 # chunk length
NCH = T // L            # 8 chunks
CHEB_DAMP = 0.25
EPS = 1e-6
QSCALE = HK ** -0.5

_PROG_CACHE = {}


def _build_program(dbg=False, reps=1):
    import concourse.bacc as bacc
    import concourse.mybir as mybir
    from concourse.tile import TileContext

    dt = mybir.dt
    f32 = dt.float32
    f16 = dt.float16
    AF = mybir.ActivationFunctionType
    ALU = mybir.AluOpType
    X = mybir.AxisListType.X

    nc = bacc.Bacc("TRN2", target_bir_lowering=False, debug=False,
                   num_devices=NCORES)

    xT16 = nc.dram_tensor("xT16", [D, T], f16, kind="ExternalInput")
    wcat = nc.dram_tensor("wcat", [D, 256], f16, kind="ExternalInput")
    wcv = nc.dram_tensor("wcv", [256, KW], f32, kind="ExternalInput")
    wg = nc.dram_tensor("wg", [D, 5], f16, kind="ExternalInput")
    wo = nc.dram_tensor("wo", [128, D], f16, kind="ExternalInput")
    alog = nc.dram_tensor("alog", [1, 2], f32, kind="ExternalInput")
    dtb5 = nc.dram_tensor("dtb5", [1, 5], f16, kind="ExternalInput")
    iden = nc.dram_tensor("iden", [128, 128], f32, kind="ExternalInput")
    iden16 = nc.dram_tensor("iden16", [128, 128], f16, kind="ExternalInput")
    umask = nc.dram_tensor("umask", [128, 128], f32, kind="ExternalInput")
    nmask = nc.dram_tensor("nmask", [128, 128], f32, kind="ExternalInput")
    outp = nc.dram_tensor("outp", [T, D], f32, kind="ExternalOutput")
    if dbg:
        dqkv = nc.dram_tensor("dqkv", [2, 128, T], f16, kind="ExternalOutput")
        dgate = nc.dram_tensor("dgate", [NCH, 128, 5], f32, kind="ExternalOutput")
        dkvtm = nc.dram_tensor("dkvtm", [NCH, 128, 128], f16, kind="ExternalOutput")
        dgt = nc.dram_tensor("dgt", [NCH, 128, 256], f16, kind="ExternalOutput")
        dhm = nc.dram_tensor("dhm", [NCH, 2, 64, 128], f32, kind="ExternalOutput")
        don = nc.dram_tensor("don", [NCH, 128, 128], f16, kind="ExternalOutput")

    with TileContext(nc) as tc:
      import contextlib
      for _rep in range(reps):
        ctx = contextlib.ExitStack()
        with ctx:
            pers = ctx.enter_context(tc.tile_pool(name="pers", bufs=1))
            p_gl = ctx.enter_context(tc.tile_pool(name="p_gl", bufs=8))
            p_big = ctx.enter_context(tc.tile_pool(name="p_big", bufs=6))
            p_gt = ctx.enter_context(tc.tile_pool(name="p_gt", bufs=9))
            p_gb = ctx.enter_context(tc.tile_pool(name="p_gb", bufs=9))
            p_sm = ctx.enter_context(tc.tile_pool(name="p_sm", bufs=9))
            p_kv = ctx.enter_context(tc.tile_pool(name="p_kv", bufs=9))
            p_hm = ctx.enter_context(tc.tile_pool(name="p_hm", bufs=10))
            p_xq = ctx.enter_context(tc.tile_pool(name="p_xq", bufs=8))
            p_out = ctx.enter_context(tc.tile_pool(name="p_out", bufs=4))
            ps_all = ctx.enter_context(tc.tile_pool(name="ps_all", bufs=8, space="PSUM"))
            ps_pj = ps_big = ps_med = ps_sm = ps_all

            # ---- persistent loads (xt split per d-tile so compute starts early) ----
            wcat_sb = pers.tile([128, 8, 256], f16)
            nc.sync.dma_start(out=wcat_sb[:], in_=wcat[:].rearrange("(a p) c -> p a c", p=128))
            wcv_sb = pers.tile([128, 2, KW], f32)
            nc.sync.dma_start(out=wcv_sb[:], in_=wcv[:].rearrange("(a p) k -> p a k", p=128))
            wg_sb = pers.tile([128, 8, 5], f16)
            nc.sync.dma_start(out=wg_sb[:], in_=wg[:].rearrange("(a p) c -> p a c", p=128))
            xt16_sb = pers.tile([128, 8, T], f16)
            for d in range(8):
                nc.sync.dma_start(out=xt16_sb[:, d, :], in_=xT16[d * 128:(d + 1) * 128, :])
            wo_sb = pers.tile([128, D], f16)
            nc.sync.dma_start(out=wo_sb[:], in_=wo[:])
            alog_sb = pers.tile([1, 2], f32)
            nc.sync.dma_start(out=alog_sb[:], in_=alog[:])
            dtb5_sb = pers.tile([1, 5], f16)
            nc.sync.dma_start(out=dtb5_sb[:], in_=dtb5[:])
            iden_sb = pers.tile([128, 128], f32)
            nc.sync.dma_start(out=iden_sb[:], in_=iden[:])
            iden16_sb = pers.tile([128, 128], f16)
            nc.sync.dma_start(out=iden16_sb[:], in_=iden16[:])
            um_sb = pers.tile([128, 128], f32)
            nc.sync.dma_start(out=um_sb[:], in_=umask[:])
            nm_sb = pers.tile([128, 128], f32)
            nc.sync.dma_start(out=nm_sb[:], in_=nmask[:])

            ones128 = pers.tile([1, 128], f32)
            nc.vector.memset(ones128[:], 1.0)
            ones128h = pers.tile([1, 128], f16)
            nc.vector.memset(ones128h[:], 1.0)
            zeros_hm = pers.tile([64, 128], f32)
            nc.vector.memset(zeros_hm[:], 0.0)
            zeros_hm16 = pers.tile([64, 128], f16)
            nc.vector.memset(zeros_hm16[:], 0.0)
            # q is used UNSCALED (no HK^-0.5): the output is linear in q and
            # the per-head rmsnorm absorbs a global scale exactly, provided the
            # rms eps is scaled by (1/QSCALE)^2 = HK.
            epsb = pers.tile([128, 1], f32)
            nc.vector.memset(epsb[:], EPS * HK)

            Rq = pers.tile([128, T], f16)     # raw q projection (pre-conv), fm
            Rkv = pers.tile([128, T], f16)    # raw k|v projection, fm
            Cq = pers.tile([128, T], f16)
            Ckv = pers.tile([128, T], f16)
            Sq = pers.tile([128, T], f16)     # silu(conv(q)), fm
            Skv = pers.tile([128, T], f16)    # silu(conv(k|v)), fm
            qst = pers.tile([64, 2 * T], f16)  # q heads stacked per chunk, scaled

            # ---- q/k/v projections (fp16): qkvT[c, t] = sum_d W[d, c] xT[d, t] ----
            for ct, dst in ((1, Rkv), (0, Rq)):
                c0 = ct * 128
                for th in range(2):
                    ps = ps_pj.tile([128, 512], f32, tag="ps")
                    for d in range(8):
                        nc.tensor.matmul(
                            ps[:],
                            wcat_sb[:, d, c0:c0 + 128],
                            xt16_sb[:, d, th * 512:(th + 1) * 512],
                            start=(d == 0), stop=(d == 7),
                        )
                    nc.vector.tensor_copy(dst[:, th * 512:(th + 1) * 512], ps[:])

            # ---- causal depthwise conv + silu; kv tile on DVE (critical for the
            # chunk preludes), q tile on the otherwise-idle GpSimd ----
            for ct, (R, C, S, eng) in ((1, (Rkv, Ckv, Skv, nc.vector)),
                                       (0, (Rq, Cq, Sq, nc.vector))):
                w = lambda k: wcv_sb[:, ct, k:k + 1]
                eng.tensor_scalar(C[:, 0:T], R[:, 0:T], w(3), None, ALU.mult)
                for tap, sh in ((2, 1), (1, 2), (0, 3)):
                    eng.scalar_tensor_tensor(
                        C[:, sh:T], R[:, 0:T - sh], w(tap), C[:, sh:T],
                        op0=ALU.mult, op1=ALU.add)
                nc.scalar.activation(S[:], C[:], AF.Silu)

            # ---- per-head gate constants broadcast over 128 partitions ----
            era = pers.tile([1, 2], f32)
            nc.scalar.activation(era[:], alog_sb[:], AF.Exp)
            ps_bc = ps_sm.tile([128, 128], f32, tag="ps")
            nc.tensor.matmul(ps_bc[:, 0:2], ones128[:], era[:], start=True, stop=True)
            negea_bc = pers.tile([128, 2], f32)
            nc.vector.tensor_scalar(negea_bc[:], ps_bc[:, 0:2], -1.0, None, ALU.mult)

            # ---- stacked, scaled q:  qst[f, ci*256 + h*128 + t] ----
            qv = qst[:].rearrange("p (c h t) -> p c h t", c=NCH, h=2)
            for h in range(2):
                nc.gpsimd.tensor_copy(
                    qv[:, :, h, :],
                    Sq[h * 64:(h + 1) * 64, :].rearrange("p (c t) -> p c t", c=NCH))

            if dbg and _rep == 0:
                nc.sync.dma_start(out=dqkv[0], in_=Sq[:])
                nc.sync.dma_start(out=dqkv[1], in_=Skv[:])

            # ---- chunked recurrence: pass 1 computes every chunk's gates,
            # decay masks, normalized k/v and the (cheap, serial) state chain;
            # pass 2 then streams all chunks' operator applications ----
            hm_prev = [zeros_hm, zeros_hm]
            hm16_prev = [zeros_hm16, zeros_hm16]
            hm16_states = []   # per chunk: incoming fp16 [H|M] per head
            chunk_ctx = []
            for grp in (range(0, 4), range(4, 8)):
                grp = list(grp)
                sl = {ci: slice(ci * L, (ci + 1) * L) for ci in grp}
                d_ = {}
                def step(nm, ci, pool, shape, dt_, ptag=None):
                    t = pool.tile(shape, dt_, tag=ptag or nm, name=f"{nm}{ci}")
                    d_.setdefault(nm, {})[ci] = t
                    return t
                # gate projections (time-major, fp16 x, dt_bias folded in)
                for ci in grp:
                    g5 = step("g5", ci, ps_sm, [128, 128], f32, "ps")
                    for d in range(8):
                        nc.tensor.matmul(g5[:, 0:5], xt16_sb[:, d, sl[ci]], wg_sb[:, d, :],
                                         start=(d == 0), stop=False)
                    nc.tensor.matmul(g5[:, 0:5], ones128h[:], dtb5_sb[:],
                                     start=False, stop=True)
                    if dbg and _rep == 0:
                        gtm = step("gtm", ci, p_sm, [128, 5], f32)
                        nc.vector.tensor_copy(gtm[:], g5[:, 0:5])
                        nc.sync.dma_start(out=dgate[ci], in_=gtm[:])
                for ci in grp:
                    nc.scalar.activation(step("e_a", ci, p_sm, [128, 2], f32)[:],
                                         d_["g5"][ci][:, 0:2], AF.Exp)
                for ci in grp:
                    nc.scalar.activation(step("sp_tm", ci, p_sm, [128, 2], f32)[:],
                                         d_["e_a"][ci][:], AF.Ln, bias=1.0)
                for ci in grp:
                    gp = step("g_pad", ci, p_sm, [128, 33], f32)
                    nc.vector.tensor_tensor(gp[:, 0:1], d_["sp_tm"][ci][:, 0:1],
                                            negea_bc[:, 0:1], ALU.mult)
                    nc.vector.tensor_tensor(gp[:, 32:33], d_["sp_tm"][ci][:, 1:2],
                                            negea_bc[:, 1:2], ALU.mult)
                for ci in grp:
                    nc.scalar.activation(step("e_g", ci, p_sm, [128, 3], f32)[:],
                                         d_["g5"][ci][:, 2:5], AF.Exp, scale=-1.0)
                for ci in grp:
                    nc.vector.tensor_scalar(step("d_g", ci, p_sm, [128, 3], f32)[:],
                                            d_["e_g"][ci][:], 1.0, None, ALU.add)
                for ci in grp:
                    nc.vector.reciprocal(step("ab_tm", ci, p_sm, [128, 3], f32)[:],
                                         d_["d_g"][ci][:])
                for ci in grp:
                    psG = step("psG", ci, ps_sm, [128, 128], f32, "ps")
                    nc.tensor.matmul(psG[:, 0:33], um_sb[:], d_["g_pad"][ci][:],
                                     start=True, stop=True)
                    psGr = step("psGr", ci, ps_sm, [128, 128], f32, "ps")
                    nc.tensor.matmul(psGr[0:33, :], d_["g_pad"][ci][:], um_sb[:],
                                     start=True, stop=True)
                for ci in grp:
                    G_sb = step("G_sb", ci, p_sm, [128, 2], f32)
                    nc.vector.tensor_copy(G_sb[:, 0:1], d_["psG"][ci][:, 0:1])
                    nc.vector.tensor_copy(G_sb[:, 1:2], d_["psG"][ci][:, 32:33])
                    grow = step("grow", ci, p_gl, [1, 256], f32)
                    nc.vector.tensor_copy(grow[0:1, 0:L], d_["psGr"][ci][0:1, :])
                    nc.vector.tensor_copy(grow[0:1, L:2 * L], d_["psGr"][ci][32:33, :])
                for ci in grp:
                    nc.scalar.activation(step("gamrow", ci, p_gl, [1, 256], f16)[:],
                                         d_["grow"][ci][:], AF.Exp)
                for ci in grp:
                    psGB = step("psGB", ci, ps_big, [128, 256], f32, "ps")
                    nc.tensor.matmul(psGB[:], ones128[:], d_["grow"][ci][:],
                                     start=True, stop=True)
                for ci in grp:
                    dm2 = step("dm2", ci, p_big, [128, 256], f32)
                    for h in range(2):
                        nc.vector.scalar_tensor_tensor(
                            dm2[:, h * L:(h + 1) * L], d_["psGB"][ci][:, h * L:(h + 1) * L],
                            d_["G_sb"][ci][:, h:h + 1], nm_sb[:],
                            op0=ALU.subtract, op1=ALU.min)
                for ci in grp:
                    nc.scalar.activation(step("gt", ci, p_gt, [128, 256], f16)[:],
                                         d_["dm2"][ci][:], AF.Exp)
                for ci in grp:
                    nc.scalar.activation(step("wend", ci, p_sm, [128, 2], f32)[:],
                                         d_["dm2"][ci][:].rearrange("p (a t) -> p a t", a=2)[:, :, L - 1],
                                         AF.Exp)
                for ci in grp:
                    psgb = step("psgb", ci, ps_med, [64, 256], f32, "ps")
                    nc.tensor.matmul(psgb[:], ones128h[0:1, 0:64], d_["gamrow"][ci][:],
                                     start=True, stop=True)
                for ci in grp:
                    nc.vector.tensor_copy(step("gb", ci, p_gb, [64, 256], f16)[:],
                                          d_["psgb"][ci][:])
                    nc.vector.tensor_copy(
                        step("gbL", ci, p_gb, [64, 2], f32)[:],
                        d_["psgb"][ci][:].rearrange("p (a t) -> p a t", a=2)[:, :, L - 1])
                # k/v transposes, k-norm, v beta scale
                for ci in grp:
                    pskt = step("pskt", ci, ps_sm, [128, 128], f16, "ps")
                    nc.tensor.transpose(pskt[:, 0:64], Skv[0:64, sl[ci]], iden16_sb[0:64, 0:64])
                    psvt = step("psvt", ci, ps_sm, [128, 128], f16, "ps")
                    nc.tensor.transpose(psvt[:, 0:64], Skv[64:128, sl[ci]], iden16_sb[64:128, 64:128])
                for ci in grp:
                    nc.scalar.activation(step("sqk", ci, p_sm, [128, 64], f32)[:],
                                         d_["pskt"][ci][:, 0:64], AF.Square)
                for ci in grp:
                    nc.vector.tensor_reduce(step("ssk", ci, p_sm, [128, 1], f32)[:],
                                            d_["sqk"][ci][:], X, ALU.add)
                for ci in grp:
                    nc.scalar.activation(step("lnk", ci, p_sm, [128, 1], f32)[:],
                                         d_["ssk"][ci][:], AF.Ln)
                for ci in grp:
                    nc.scalar.activation(step("nrk", ci, p_sm, [128, 1], f32)[:],
                                         d_["lnk"][ci][:], AF.Exp, scale=0.5)
                for ci in grp:
                    nc.vector.tensor_scalar(step("nre", ci, p_sm, [128, 1], f32)[:],
                                            d_["nrk"][ci][:], EPS, None, ALU.add)
                for ci in grp:
                    nc.vector.reciprocal(step("invk", ci, p_sm, [128, 1], f32)[:],
                                         d_["nre"][ci][:])
                for ci in grp:
                    kv_tm = step("kvtm", ci, p_kv, [128, 128], f16)
                    nc.vector.tensor_scalar(kv_tm[:, 0:64], d_["pskt"][ci][:, 0:64],
                                            d_["invk"][ci][:], None, ALU.mult)
                    nc.vector.tensor_scalar(kv_tm[:, 64:128], d_["psvt"][ci][:, 0:64],
                                            d_["ab_tm"][ci][:, 2:3], None, ALU.mult)
                for ci in grp:
                    pskf = step("pskf", ci, ps_sm, [128, 128], f16, "ps")
                    nc.tensor.transpose(pskf[0:64, :], d_["kvtm"][ci][:, 0:64], iden16_sb[:])
                for ci in grp:
                    nc.vector.tensor_copy(step("kfn", ci, p_kv, [64, 128], f16)[:],
                                          d_["pskf"][ci][0:64, :])
                for ci in grp:
                    kw = step("kw", ci, p_kv, [128, 128], f16)
                    for h in range(2):
                        nc.vector.tensor_scalar(kw[:, h * 64:(h + 1) * 64],
                                                d_["kvtm"][ci][:, 0:64],
                                                d_["wend"][ci][:, h:h + 1], None, ALU.mult)
                # state chain (serial across chunks, cheap)
                for ci in grp:
                    hm_new, hm16_new = [], []
                    for h in range(2):
                        ps_hm = ps_sm.tile([128, 128], f32, tag="ps", name=f"pshm{ci}_{h}")
                        nc.tensor.matmul(ps_hm[0:64, :], d_["kw"][ci][:, h * 64:(h + 1) * 64],
                                         d_["kvtm"][ci][:], start=True, stop=(ci == 0))
                        if ci > 0:
                            diag = p_hm.tile([64, 64], f32, tag="diag", name=f"diag{ci}_{h}")
                            nc.vector.tensor_scalar(diag[:], iden_sb[0:64, 0:64],
                                                    d_["gbL"][ci][:, h:h + 1], None, ALU.mult)
                            nc.tensor.matmul(ps_hm[0:64, :], diag[:], hm_prev[h][:],
                                             start=False, stop=True)
                        hm = p_hm.tile([64, 128], f32, tag="hm", name=f"hm{ci}_{h}")
                        nc.vector.tensor_copy(hm[:], ps_hm[0:64, :])
                        hm16 = p_hm.tile([64, 128], f16, tag="hm16", name=f"hm16_{ci}_{h}")
                        nc.gpsimd.tensor_copy(hm16[:], hm[:])
                        hm_new.append(hm)
                        hm16_new.append(hm16)
                        if dbg and _rep == 0:
                            nc.sync.dma_start(out=dhm[ci, h], in_=hm[:])
                    if dbg and _rep == 0:
                        nc.sync.dma_start(out=dkvtm[ci], in_=d_["kvtm"][ci][:])
                        nc.sync.dma_start(out=dgt[ci], in_=d_["gt"][ci][:])
                    hm16_states.append(hm16_prev)
                    hm16_prev = hm16_new
                    hm_prev = hm_new
                    chunk_ctx.append((d_["kvtm"][ci], d_["kfn"][ci], d_["gt"][ci],
                                      d_["gb"][ci], d_["ab_tm"][ci]))

            # ---- pass 2: operator applications + output, emitted step-major
            # over groups of 4 chunks so the static schedule pipelines the
            # cross-engine chains (PE mm -> DVE mask -> PE mm -> DVE combine)
            for grp in (range(0, 4), range(4, 8)):
                grp = list(grp)
                xcur = {ci: qst[:, ci * 256:(ci + 1) * 256] for ci in grp}
                for it in range(3):          # it 0,1: H-refine; it 2: M-output
                    xg, ps_p, a_sb = {}, {}, {}
                    for ci in grp:
                        if ci > 0:
                            xg[ci] = p_xq.tile([64, 256], f16, tag="xg", name=f"xg{ci}")
                            nc.vector.tensor_tensor(xg[ci][:], xcur[ci],
                                                    chunk_ctx[ci][3][:], ALU.mult)
                    for ci in grp:
                        ps_p[ci] = ps_big.tile([128, 256], f32, tag="ps", name=f"psp{ci}")
                        nc.tensor.matmul(ps_p[ci][:], chunk_ctx[ci][1][:], xcur[ci],
                                         start=True, stop=True)
                    for ci in grp:
                        a_sb[ci] = p_big.tile([128, 256], f16, tag="a", name=f"asb{ci}")
                        nc.vector.tensor_tensor(a_sb[ci][:], ps_p[ci][:],
                                                chunk_ctx[ci][2][:], ALU.mult)
                    if it < 2:
                        ps_y = {}
                        for ci in grp:
                            kv_tm = chunk_ctx[ci][0]
                            ps_y[ci] = ps_med.tile([64, 256], f32, tag="ps", name=f"psy{ci}")
                            nc.tensor.matmul(ps_y[ci][:], kv_tm[:, 0:64], a_sb[ci][:],
                                             start=True, stop=(ci == 0))
                            if ci > 0:
                                for h in range(2):
                                    nc.tensor.matmul(
                                        ps_y[ci][:, h * L:(h + 1) * L],
                                        hm16_states[ci][h][:, 0:64],
                                        xg[ci][:, h * L:(h + 1) * L],
                                        start=False, stop=True)
                        for ci in grp:
                            xq = p_xq.tile([64, 256], f16, tag="xq")
                            nc.vector.scalar_tensor_tensor(
                                xq[:], ps_y[ci][:], -CHEB_DAMP,
                                qst[:, ci * 256:(ci + 1) * 256],
                                op0=ALU.mult, op1=ALU.add)
                            xcur[ci] = xq[:]
                    else:
                        ps_o = {}
                        for ci in grp:
                            kv_tm = chunk_ctx[ci][0]
                            ps_o[ci] = ps_sm.tile([128, 128], f32, tag="ps", name=f"pso{ci}")
                            for h in range(2):
                                nc.tensor.matmul(
                                    ps_o[ci][:, h * 64:(h + 1) * 64],
                                    a_sb[ci][:, h * L:(h + 1) * L], kv_tm[:, 64:128],
                                    start=True, stop=(ci == 0))
                                if ci > 0:
                                    nc.tensor.matmul(
                                        ps_o[ci][:, h * 64:(h + 1) * 64],
                                        xg[ci][:, h * L:(h + 1) * L],
                                        hm16_states[ci][h][:, 64:128],
                                        start=False, stop=True)

                # ---- alpha gate + per-head rmsnorm (time-major), step-major ----
                oa, sqo, sso, lno, rmso, invo, on, ofm = {}, {}, {}, {}, {}, {}, {}, {}
                for ci in grp:
                    al2 = chunk_ctx[ci][4][:, 0:2].unsqueeze(2).broadcast_to([128, 2, 64])
                    oa[ci] = p_out.tile([128, 128], f32, tag="oa", name=f"oa{ci}")
                    nc.vector.tensor_tensor(oa[ci][:].rearrange("p (h v) -> p h v", h=2),
                                            ps_o[ci][:].rearrange("p (h v) -> p h v", h=2),
                                            al2, ALU.mult)
                for ci in grp:
                    sqo[ci] = p_out.tile([128, 128], f32, tag="sqo", name=f"sqo{ci}")
                    nc.scalar.activation(sqo[ci][:], oa[ci][:], AF.Square)
                for ci in grp:
                    sso[ci] = p_sm.tile([128, 2], f32, tag="sso", name=f"sso{ci}")
                    nc.vector.tensor_reduce(sso[ci][:],
                                            sqo[ci][:].rearrange("p (h v) -> p h v", h=2),
                                            X, ALU.add)
                for ci in grp:
                    lno[ci] = p_sm.tile([128, 2], f32, tag="lno", name=f"lno{ci}")
                    nc.scalar.activation(lno[ci][:], sso[ci][:], AF.Ln, bias=epsb[:],
                                         scale=1.0 / 64.0)
                for ci in grp:
                    rmso[ci] = p_sm.tile([128, 2], f32, tag="rmso", name=f"rmso{ci}")
                    nc.scalar.activation(rmso[ci][:], lno[ci][:], AF.Exp, scale=0.5)
                for ci in grp:
                    invo[ci] = p_sm.tile([128, 2], f32, tag="invo", name=f"invo{ci}")
                    nc.vector.reciprocal(invo[ci][:], rmso[ci][:])
                for ci in grp:
                    on[ci] = p_out.tile([128, 128], f16, tag="on", name=f"on{ci}")
                    nc.vector.tensor_tensor(
                        on[ci][:].rearrange("p (h v) -> p h v", h=2),
                        oa[ci][:].rearrange("p (h v) -> p h v", h=2),
                        invo[ci][:].unsqueeze(2).broadcast_to([128, 2, 64]), ALU.mult)
                    if dbg and _rep == 0:
                        nc.sync.dma_start(out=don[ci], in_=on[ci][:])
                ps_of, ps_out = {}, {}
                for ci in grp:
                    ps_of[ci] = ps_sm.tile([128, 128], f16, tag="ps", name=f"psof{ci}")
                    nc.tensor.transpose(ps_of[ci][:], on[ci][:], iden16_sb[:])
                for ci in grp:
                    ofm[ci] = p_out.tile([128, 128], f16, tag="ofm", name=f"ofm{ci}")
                    nc.vector.tensor_copy(ofm[ci][:], ps_of[ci][:])
                for ci in grp:
                    out_sb = p_out.tile([128, D], f32, tag="outsb")
                    for nh in range(2):
                        ps_out = ps_pj.tile([128, 512], f32, tag="ps")
                        nc.tensor.matmul(ps_out[:], ofm[ci][:],
                                         wo_sb[:, nh * 512:(nh + 1) * 512],
                                         start=True, stop=True)
                        nc.scalar.copy(out_sb[:, nh * 512:(nh + 1) * 512], ps_out[:])
                    nc.sync.dma_start(out=outp[ci * L:(ci + 1) * L, :], in_=out_sb[:])

    # The act-table placement pass maps each activation func to the FIRST
    # table containing it; Exp->exp_and_others and Ln->natural_log would then
    # thrash with a table reload on every Exp<->Ln alternation. Compile with
    # natural_log_exp_and_others (has both) hoisted to the front, then remap
    # the emitted set ids back to the real act_info.json indices.
    import concourse.bacc as bacc_mod
    from concourse.hw_specs import get_activation_tables as _gat
    orig_tables = _gat(nc.m.arch)
    orig_names = list(orig_tables.keys())
    pref = "natural_log_exp_and_others"
    reordered = {pref: orig_tables[pref],
                 **{k: v for k, v in orig_tables.items() if k != pref}}
    pnames = list(reordered.keys())
    bacc_mod.get_activation_tables = lambda arch: reordered
    try:
        nc.compile()
    finally:
        bacc_mod.get_activation_tables = _gat
    for b in nc.main_func.blocks:
        for i in b.instructions:
            if isinstance(i, mybir.InstLoadActFuncSet):
                i.act_func_set_id = orig_names.index(pnames[i.act_func_set_id])
    return nc


def _prep_core_inputs(c, x, Wq, Wk, Wv, Wconv, Wa, Walpha, Wb, A_log, dt_bias,
                      norm_w, Wo, xT, xT16, iden, iden16, um, nm):
    f32, f16 = np.float32, np.float16
    h0, h1, hk = 2 * c, 2 * c + 1, c // 2
    wbase = np.hstack([
        Wq[:, h0 * HK:(h0 + 1) * HK], Wq[:, h1 * HK:(h1 + 1) * HK],
        Wk[:, hk * HK:(hk + 1) * HK], Wv[:, hk * HV:(hk + 1) * HV],
    ]).astype(f32)
    wgm = np.hstack([
        Wa[:, h0:h0 + 1], Wa[:, h1:h1 + 1],
        Walpha[:, h0:h0 + 1], Walpha[:, h1:h1 + 1],
        Wb[:, hk:hk + 1],
    ]).astype(f16)
    qoff, koff, voff = 0, HQ * HK, HQ * HK + HKV * HK
    wcv = np.vstack([
        Wconv[qoff + h0 * HK: qoff + (h0 + 1) * HK],
        Wconv[qoff + h1 * HK: qoff + (h1 + 1) * HK],
        Wconv[koff + hk * HK: koff + (hk + 1) * HK],
        Wconv[voff + hk * HV: voff + (hk + 1) * HV],
    ]).astype(f32)
    wcat = wbase.astype(f16)
    wo_scale = np.tile(np.asarray(norm_w, f32), HQ)
    Wo_s = np.asarray(Wo, f32) * wo_scale[:, None]
    wo = np.ascontiguousarray(
        np.vstack([Wo_s[h0 * HV:(h0 + 1) * HV], Wo_s[h1 * HV:(h1 + 1) * HV]])).astype(f16)
    alog = np.asarray(A_log, f32)[[h0, h1]].reshape(1, 2).copy()
    dtbv = np.zeros((1, 5), np.float16)
    dtbv[0, 0:2] = np.asarray(dt_bias, f32)[[h0, h1]]
    return dict(xT16=xT16, wcat=np.ascontiguousarray(wcat), wg=wgm,
                wo=wo, wcv=np.ascontiguousarray(wcv), alog=alog, dtb5=dtbv,
                iden=iden, iden16=iden16, umask=um, nmask=nm)


def make_in_maps(x, Wq, Wk, Wv, Wconv, Wa, Walpha, Wb, A_log, dt_bias, norm_w, Wo):
    f32, f16 = np.float32, np.float16
    x2 = np.asarray(x, f32).reshape(T, D)
    xT = np.ascontiguousarray(x2.T)
    xT16 = xT.astype(f16)
    iden = np.eye(128, dtype=f32)
    iden16 = np.eye(128, dtype=f16)
    um = np.ascontiguousarray(np.triu(np.ones((128, 128), f32)))
    nm = np.ascontiguousarray(np.where(um > 0, 0.0, -30000.0).astype(f32))
    args = (x, np.asarray(Wq, f32), np.asarray(Wk, f32), np.asarray(Wv, f32),
            np.asarray(Wconv, f32), np.asarray(Wa, f32), np.asarray(Walpha, f32),
            np.asarray(Wb, f32), A_log, dt_bias, norm_w, Wo)
    return [_prep_core_inputs(c, *args, xT=xT, xT16=xT16, iden=iden,
                              iden16=iden16, um=um, nm=nm)
            for c in range(NCORES)]


def get_program(dbg=False, reps=1):
    key = (dbg, reps)
    if key not in _PROG_CACHE:
        _PROG_CACHE[key] = _build_program(dbg, reps)
    return _PROG_CACHE[key]


def kernel(**inputs) -> np.ndarray:
    from concourse.bass_utils import run_bass_kernel_spmd
    nc = get_program(dbg=False)
    in_maps = make_in_maps(**inputs)
    res = run_bass_kernel_spmd(nc, in_maps, list(range(NCORES)))
    out = np.zeros((T, D), np.float32)
    for c in range(NCORES):
        out += res.results[c]["outp"]
    return out.reshape(B, T, D)
